# revision 20
# baseline (speedup 1.0000x reference)
"""Trainium2 Bass kernel for nn_MultiHeadAttention_56118042690041.

8-core sharding: batch x heads tensor-parallel.
  core c (0..7): batch b = c//4, heads 4*(c%4) .. 4*(c%4)+4 (as 2 packed pairs).

v3 pipeline (all-bf16 dataflow, fp32 PSUM accumulation):
  - x and all weights cast to bf16 AND pre-packed on host into the exact SBUF
    layouts ([part, ktile*width+col]) so every DMA is one large contiguous
    transfer (the v2 kernel burned ~45us issuing ~200 small DMAs through the
    sync sequencer).
  - v/k projections: W-stationary matmuls, bias (+1/8 scale for k) on DVE,
    outputs kept bf16 in SBUF as [128(dk pair), S].
  - q projection emitted TRANSPOSED (x-stationary, W moving) directly into the
    [t, dk] layout AV needs; the per-head ones column (softmax denominator
    trick) comes from memsetting the qa tiles to 1.0.
  - attention: slot pipeline over (head, t-block, s-half). Per slot the PE runs
    2 score matmuls (K=64) into a double-buffered PSUM tile and 2 AV matmuls of
    the PREVIOUS head (K=128, tk-outer so exp tiles free early), while ACT runs
    exp back-to-back.
  - softmax normalize: reciprocal of the denominator row (bf16), PE K=1 matmul
    broadcasts it into rows 64:128 of the same av PSUM tile, DVE copy to SBUF,
    multiply + bq add -> headout bf16.
  - collectives: pair-0 headout AllGather'd (bf16) early in head 3's slots;
    pair-1 gathered in four 512-column slices, each fired right after its
    slice of the last head is normalized, so the output projection pipelines
    behind the collective slice by slice.
  - output projection interleaved into the last head's AV: per s-block,
    pair-0 blocks (gathered long ago) accumulate first, pair-1 blocks last.
"""

import contextlib
import ctypes
import os
import sys
import types

import ml_dtypes
import numpy as np

if "/opt/trn_rl_repo" not in sys.path:
    sys.path.insert(0, "/opt/trn_rl_repo")

# ---------------------------------------------------------------- shims ----


def _install_antenv_shim():
    """Provide antenv.axon_hooks (NTFF profile hook) if the image lacks it."""
    try:
        import antenv.axon_hooks  # noqa: F401

        return
    except ImportError:
        pass

    def _hook_factory():
        so_path = "/opt/axon/libaxon_pjrt.so"
        try:
            lib = ctypes.CDLL(so_path)
        except OSError:
            return None
        if not hasattr(lib, "axon_start_nrt_profile"):
            return None
        lib.axon_start_nrt_profile.argtypes = [
            ctypes.POINTER(ctypes.c_int64),
            ctypes.c_size_t,
        ]
        lib.axon_start_nrt_profile.restype = ctypes.c_int64
        lib.axon_stop_nrt_profile.argtypes = [ctypes.c_char_p]
        lib.axon_stop_nrt_profile.restype = ctypes.c_int64

        @contextlib.contextmanager
        def _hook(output_dir, device_ids):
            import jax

            jax.devices()
            if device_ids:
                ids = (ctypes.c_int64 * len(device_ids))(*device_ids)
                rc = lib.axon_start_nrt_profile(ids, len(device_ids))
            else:
                rc = lib.axon_start_nrt_profile(None, 0)
            if rc != 0:
                raise RuntimeError(f"axon_start_nrt_profile rc={rc}")
            try:
                yield
            finally:
                n = lib.axon_stop_nrt_profile(str(output_dir).encode())
                print(f"ntff profile: {n} file(s) -> {output_dir}", file=sys.stderr)

        return _hook

    hook = _hook_factory()
    mod = types.ModuleType("antenv.axon_hooks")
    mod.get_axon_ntff_profile_hook = lambda: hook
    mod.set_axon_ntff_profile_hook = lambda h: None
    sys.modules["antenv.axon_hooks"] = mod


def _install_tile_drain_patch():
    """This walrus build rejects >1 sync wait on the Tile tail Drain; split the
    waits across chained single-wait drains."""
    import concourse.tile as tile

    if getattr(tile.TileContext, "_drain_patch_installed", False):
        return

    def _drain_and_barrier(self, tick_clock, wait_clock):
        nc = self.nc
        drain_inst = nc.sync.drain()
        wait_clock.add_sem_waits(
            drain_inst.ins, tile.ScopedClock({None: tick_clock.global_clock})
        )
        si = drain_inst.ins.sync_info
        waits = list(si.on_wait) if si is not None and si.on_wait else []
        if len(waits) > 1:
            si.on_wait = waits[:1]
            assert self.sems is not None
            by_num = {h.num: h for h in self.sems.allocated().values()}
            for w in waits[1:]:
                d2 = nc.sync.drain()
                h = by_num.get(w.id)
                assert h is not None, f"no sem handle for wait {w.ant_name}"
                d2.wait_op(h, w.wait_value, "sem-ge", check=False)
        nc.all_engine_barrier()
        assert self.sems is not None
        popped = nc._tile_sem_poison_stack.pop()
        assert popped is self._sem_poison
        nc.clear_and_free_semaphores(list(self.sems.allocated().values()))
        nc.all_engine_barrier()

    tile.TileContext._drain_and_barrier = _drain_and_barrier
    tile.TileContext._drain_patch_installed = True


_install_antenv_shim()


def _split_multi_waits(nc, max_waits=1):
    """This walrus build rejects instructions carrying more than ~1 sync wait.
    Move excess waits onto same-engine NOPs inserted immediately before the
    instruction (sequencer waits execute in stream order, so this is
    semantics-preserving)."""
    import bass_rust
    import concourse.mybir as mybir

    n = 0
    for bb in nc.m.functions[0].blocks:
        insts = bb.instructions
        out = []
        for inst in insts:
            si = inst.sync_info
            waits = list(si.on_wait) if si is not None and si.on_wait else []
            if len(waits) > max_waits:
                keep = waits[-max_waits:]
                for w in waits[:-max_waits]:
                    nop = mybir.InstNoOp(name=f"waitnop_{n}", ins=[], outs=[])
                    n += 1
                    nop.engine = inst.engine
                    nop.sync_info = bass_rust.SyncInfo(on_wait=[w], on_update=[])
                    out.append(nop)
                si.on_wait = keep
            out.append(inst)
        if len(out) != len(insts):
            insts[:] = out
    return n


# ------------------------------------------------------------- program -----

N_CORES = 8
GROUP = 4  # cores per batch group

last_results = None  # BassKernelResults of the most recent run (for test.py)


def build_program(S=2048, DM=1024, H=16, DK=64, split_waits=True):
    """Emit the SPMD Bass/Tile program. Returns nc."""
    import concourse.bass as bass
    import concourse.mybir as mybir
    import concourse.tile as tile

    _install_tile_drain_patch()

    f32 = mybir.dt.float32
    bf16 = mybir.dt.bfloat16
    NPAIR = 2  # head pairs per core (4 heads)
    NH = 2 * NPAIR  # heads per core
    KT = DM // 128  # contraction tiles for projections
    TT = S // 128  # t-blocks
    NS2 = S // 512  # AV s-blocks
    NTS = S // 512  # x strips
    HDK = H * DK
    KO = HDK // 128  # outproj contraction tiles
    DSL = HDK // GROUP  # out d-slice per core (256)

    nc = bass.Bass(
        trn_type="TRN2", target_bir_lowering=False, debug=False, num_devices=N_CORES
    )

    def din(name, shape, dt=bf16):
        return nc.dram_tensor(name, shape, dt, kind="ExternalInput").ap()

    # host-packed: [128, NTS, KT, 512] flattened -> strip ts is one slice
    xP = {p: din(f"x{p}P", [128, NTS * KT * 512]) for p in ("q", "k", "v")}
    # host-packed W.T: [p][part, kk*128+col]
    W = {p: din(f"w{p}P", [NPAIR, 128, KT * 128]) for p in ("k", "v")}
    wq4 = din("wq4P", [128, KT * 256])  # all 4 local heads per ktile
    bq = din("bq", [NPAIR, 128, 1], f32)
    bk8 = din("bk8", [NPAIR, 128, 1], f32)  # bk / sqrt(dk)
    bv = din("bv", [NPAIR, 128, 1], f32)
    woT = din("woTP", [128, KO * DSL])  # host-packed Wo.T slice
    boT = din("boT", [128, 2], f32)  # bo d-slice as [128, 2]
    out_ap = nc.dram_tensor("out", [DSL, S], f32, kind="ExternalOutput").ap()

    with tile.TileContext(nc) as tc:
        with contextlib.ExitStack() as ctx:
            sb = ctx.enter_context(tc.tile_pool(name="sb", bufs=2))
            big = ctx.enter_context(tc.tile_pool(name="big", bufs=8))
            ps = ctx.enter_context(tc.tile_pool(name="ps", bufs=2, space="PSUM"))
            dram = ctx.enter_context(tc.tile_pool(name="dram", bufs=1, space="DRAM"))

            # --- weights for v/k first (unblock projections ASAP) ---
            w_sb = {}
            for kind in ("v", "k"):
                w_sb[kind] = [
                    sb.tile([128, KT * 128], bf16, tag="w", bufs=4, name=f"w_{kind}{p}")
                    for p in range(NPAIR)
                ]
                for p in range(NPAIR):
                    nc.sync.dma_start(w_sb[kind][p][:], W[kind][p])
            bq_sb = sb.tile([128, NPAIR], f32, tag="bq", bufs=1)
            bk_sb = sb.tile([128, NPAIR], f32, tag="bk", bufs=1)
            bv_sb = sb.tile([128, NPAIR], f32, tag="bv", bufs=1)
            for p in range(NPAIR):
                nc.sync.dma_start(bq_sb[:, p : p + 1], bq[p])
                nc.sync.dma_start(bk_sb[:, p : p + 1], bk8[p])
                nc.sync.dma_start(bv_sb[:, p : p + 1], bv[p])

            # --- phase P: v/k projections -> pair tiles [128, S] bf16 ---
            proj_out = {}
            for kind in ("v", "k"):
                outs = [
                    big.tile([128, S], bf16, tag="vk", bufs=4, name=f"{kind}2T_{p}")
                    for p in range(NPAIR)
                ]
                proj_out[kind] = outs
                for ts in range(NTS):
                    xt = sb.tile([128, KT * 512], bf16, tag="xt", bufs=2, name="xt")
                    nc.sync.dma_start(
                        xt[:], xP[kind][:, ts * KT * 512 : (ts + 1) * KT * 512]
                    )
                    prs = [
                        ps.tile(
                            [128, 512], f32, tag="ps512", bufs=4, name=f"pr{kind}{p}"
                        )
                        for p in range(NPAIR)
                    ]
                    for kk in range(KT):
                        for p in range(NPAIR):
                            nc.tensor.matmul(
                                prs[p][:],
                                w_sb[kind][p][:, kk * 128 : (kk + 1) * 128],
                                xt[:, kk * 512 : (kk + 1) * 512],
                                start=(kk == 0),
                                stop=(kk == KT - 1),
                            )
                    for p in range(NPAIR):
                        dst = outs[p][:, ts * 512 : (ts + 1) * 512]
                        if kind == "k":
                            nc.vector.tensor_scalar(
                                dst,
                                prs[p][:],
                                1.0 / 8.0,
                                bk_sb[:, p : p + 1],
                                mybir.AluOpType.mult,
                                mybir.AluOpType.add,
                            )
                        else:
                            nc.vector.tensor_scalar_add(
                                dst, prs[p][:], bv_sb[:, p : p + 1]
                            )
                if kind == "v":
                    # late-needed consts while v-proj streams
                    ones64 = sb.tile([1, 64], bf16, tag="ones", bufs=1)
                    nc.gpsimd.memset(ones64[:], 1.0)
                    wq_sb = sb.tile([128, KT * 256], bf16, tag="wq", bufs=1)
                    nc.sync.dma_start(wq_sb[:], wq4[:])
            v2T, k2T = proj_out["v"], proj_out["k"]
            woT_sb = sb.tile([128, KO * DSL], bf16, tag="wo", bufs=1)
            nc.sync.dma_start(woT_sb[:], woT[:])
            boT_sb = sb.tile([128, 2], f32, tag="bo", bufs=1)
            nc.sync.dma_start(boT_sb[:], boT[:])

            # qa tiles: per pair [128, TT*130] layout per t-block:
            #   [headA 64 | onesA 1 | headB 64 | onesB 1]
            qa = [
                big.tile([128, TT * 130], bf16, tag="qa", bufs=NPAIR, name=f"qa{p}")
                for p in range(NPAIR)
            ]
            for p in range(NPAIR):
                nc.gpsimd.memset(qa[p][:], 1.0)

            # --- phase A: slot-pipelined attention + transposed q-proj ---
            headout = [
                big.tile([128, S], bf16, tag="ho", bufs=NPAIR, name=f"headout_{p}")
                for p in range(NPAIR)
            ]
            cc_in0 = dram.tile([128, S], bf16, name="cc_in0")
            cc_out0 = dram.tile([GROUP * 128, S], bf16, name="cc_out0")
            cc_in1s = [
                dram.tile([128, 512], bf16, name=f"cc_in1_{j}") for j in range(NS2)
            ]
            cc_out1s = [
                dram.tile([GROUP * 128, 512], bf16, name=f"cc_out1_{j}")
                for j in range(NS2)
            ]

            ET_BUFS = 20
            pend = []  # deferred norm pieces, flushed at the next slot start
            et_tiles = {}  # (h, tb) -> tile (fresh ring allocation per head)

            def et_get(h, tb):
                key = (h, tb)
                if key not in et_tiles:
                    et_tiles[key] = big.tile(
                        [128, S], bf16, tag="et", bufs=ET_BUFS, name=f"et{h}_{tb}"
                    )
                return et_tiles[key]

            def emit_scores(h, tb, half, sc):
                p, prow = h // 2, 64 * (h % 2)
                for j in range(2):
                    nc.tensor.matmul(
                        sc[:, j * 512 : (j + 1) * 512],
                        k2T[p][prow : prow + 64, tb * 128 : (tb + 1) * 128],
                        v2T[p][
                            prow : prow + 64,
                            half * 1024 + j * 512 : half * 1024 + (j + 1) * 512,
                        ],
                        start=True,
                        stop=True,
                    )

            def emit_qproj(tb, half, qp, xq_tiles):
                # accumulate Wq over 4 of 8 ktiles into qp[:, :256]
                g, sub = tb // 4, tb % 4
                for kk in range(half * 4, half * 4 + 4):
                    nc.tensor.matmul(
                        qp[:, 0:256],
                        xq_tiles[g][:, kk * 512 + sub * 128 : kk * 512 + (sub + 1) * 128],
                        wq_sb[:, kk * 256 : (kk + 1) * 256],
                        start=(kk == 0),
                        stop=(kk == KT - 1),
                    )

            def emit_qcopy(tb, qp):
                for hh in range(NH):
                    p = hh // 2
                    dst = qa[p][
                        :, tb * 130 + 65 * (hh % 2) : tb * 130 + 65 * (hh % 2) + 64
                    ]
                    nc.vector.tensor_copy(dst, qp[:, hh * 64 : (hh + 1) * 64])

            def emit_av(hprev, tk, s2, av):
                qoff = 65 * (hprev % 2)
                nc.tensor.matmul(
                    av[0:65, :],
                    qa[hprev // 2][:, tk * 130 + qoff : tk * 130 + qoff + 65],
                    et_get(hprev, tk)[:, s2 * 512 : (s2 + 1) * 512],
                    start=(tk == 0),
                    stop=(tk == TT - 1),
                )
                if tk == TT - 1:
                    emit_norm_a(hprev, s2, av)

            def emit_norm_a(h, s2, av):
                # reciprocal of denominator row (DVE), then defer bc + mul
                rcp1 = sb.tile([1, 512], bf16, tag="rcp", bufs=4, name=f"rcp{s2 % 4}")
                with nc.allow_low_precision(reason="1/den in bf16 for PE broadcast"):
                    nc.vector.reciprocal(rcp1[:], av[64:65, :])
                pend.append((h, s2, av, rcp1))

            def flush_pend():
                while pend:
                    h, s2, av, rcp1 = pend.pop(0)
                    p, prow = h // 2, 64 * (h % 2)
                    # broadcast 1/den into rows 64:128 of the same PSUM tile
                    nc.tensor.matmul(
                        av[64:128, :], ones64[:], rcp1[:], start=True, stop=True
                    )
                    # DVE tensor_tensor cannot take two PSUM operands; stage
                    # the broadcast through SBUF.
                    bcs = sb.tile([64, 512], f32, tag="bcs", bufs=4, name=f"bcs{s2 % 4}")
                    nc.vector.tensor_copy(bcs[:], av[64:128, :])
                    dst = headout[p][prow : prow + 64, s2 * 512 : (s2 + 1) * 512]
                    nc.vector.tensor_mul(dst, av[0:64, :], bcs[:])
                    nc.vector.tensor_scalar_add(
                        dst, dst, bq_sb[prow : prow + 64, p : p + 1]
                    )

            xq_tiles = {}
            av_cur = None
            for h in range(NH):
                # av tiles for the PREVIOUS head's AV accumulation (tk-outer:
                # all four s2 accumulations open at once, exp tiles free early)
                if h >= 1:
                    av_cur = [
                        ps.tile([128, 512], f32, tag="ps512", bufs=4, name=f"av{s2}")
                        for s2 in range(NS2)
                    ]
                for tb in range(TT):
                    if h == 0:
                        if tb % 4 == 0:
                            g = tb // 4
                            t = sb.tile(
                                [128, KT * 512], bf16, tag="xt", bufs=2, name="xt"
                            )
                            nc.sync.dma_start(
                                t[:], xP["q"][:, g * KT * 512 : (g + 1) * KT * 512]
                            )
                            xq_tiles[g] = t
                        qp = ps.tile([128, 512], f32, tag="ps512", bufs=4, name="qp")
                    et = et_get(h, tb)
                    for half in range(2):
                        flush_pend()  # previous slot's norms (bc + mul + bias)
                        if h == 0:
                            emit_qproj(tb, half, qp, xq_tiles)
                        else:
                            step = tb * 2 + half
                            for m in (2 * step, 2 * step + 1):
                                emit_av(h - 1, m // NS2, m % NS2, av_cur[m % NS2])
                        sc = ps.tile([128, 1024], f32, tag="sc", bufs=2, name="sc")
                        emit_scores(h, tb, half, sc)
                        nc.scalar.activation(
                            et[:, half * 1024 : (half + 1) * 1024],
                            sc[:],
                            mybir.ActivationFunctionType.Exp,
                        )
                    if h == 0:
                        emit_qcopy(tb, qp)
                    # AG0 fires early in head 3's slots: by then AV(1) norms
                    # (flushed at tb==0) completed headout[0] (= pair 0).
                    if h == 3 and tb == 1:
                        nc.sync.dma_start(cc_in0[:], headout[0][:])
                        nc.gpsimd.collective_compute(
                            "AllGather",
                            mybir.AluOpType.bypass,
                            replica_groups=[[0, 1, 2, 3], [4, 5, 6, 7]],
                            ins=[cc_in0.opt()],
                            outs=[cc_out0.opt()],
                        )

            # --- tail: AV of the last head (s2-inner) + sliced pair-1 AG +
            # interleaved output projection ---
            korder = [0, 2, 4, 6, 1, 3, 5, 7]  # pair0 blocks first
            AP = bass.AP

            def ch_dma(sblk, pair):
                # one DMA per (sblk, pair): 4 gathered 128-row blocks
                ch = sb.tile(
                    [128, 4 * 512], bf16, tag="ch", bufs=3, name=f"ch{pair}"
                )
                if pair == 0:
                    base = cc_out0[:]
                    src = AP(
                        base.tensor,
                        base.offset + sblk * 512,
                        [[S, 128], [128 * S, 4], [1, 512]],
                    )
                else:
                    base = cc_out1s[sblk][:]
                    src = AP(
                        base.tensor,
                        base.offset,
                        [[512, 128], [128 * 512, 4], [1, 512]],
                    )
                nc.sync.dma_start(ch[:], src)
                return ch

            ostate = {}

            def outproj_quarter(sblk, quarter):
                if quarter == 0:
                    pos = [
                        ps.tile([128, 512], f32, tag="ps512", bufs=4, name=f"po{d}")
                        for d in range(2)
                    ]
                    ostate[sblk] = [pos, ch_dma(sblk, 0), None]
                pos, ch0, ch1 = ostate[sblk]
                if quarter == 2:
                    ch1 = ch_dma(sblk, 1)
                    ostate[sblk][2] = ch1
                for i in range(2):
                    ki = quarter * 2 + i
                    k = korder[ki]
                    ch = ch0 if ki < 4 else ch1
                    rhs = ch[:, (ki % 4) * 512 : (ki % 4 + 1) * 512]
                    for dblk in range(2):
                        nc.tensor.matmul(
                            pos[dblk][:],
                            woT_sb[
                                :, k * DSL + 128 * dblk : k * DSL + 128 * (dblk + 1)
                            ],
                            rhs,
                            start=(ki == 0),
                            stop=(ki == KO - 1),
                        )
                if quarter == 3:
                    for dblk in range(2):
                        ob = sb.tile([128, 512], f32, tag="ob", bufs=3, name="ob")
                        nc.vector.tensor_scalar_add(
                            ob[:], pos[dblk][:], boT_sb[:, dblk : dblk + 1]
                        )
                        nc.sync.dma_start(
                            out_ap[
                                128 * dblk : 128 * (dblk + 1),
                                sblk * 512 : (sblk + 1) * 512,
                            ],
                            ob[:],
                        )
                    del ostate[sblk]

            for s2 in range(NS2):
                av = ps.tile([128, 512], f32, tag="ps512", bufs=4, name=f"avt{s2 % 2}")
                for tkq in range(4):
                    flush_pend()
                    if s2 >= 1:
                        outproj_quarter(s2 - 1, tkq)
                    for tk in range(tkq * 4, tkq * 4 + 4):
                        emit_av(NH - 1, tk, s2, av)
                flush_pend()
                # pair-1 AG slice: heads 2 (normalized during h=3 slots) and 3
                # (just normalized) for this 512-column block
                nc.sync.dma_start(
                    cc_in1s[s2][:], headout[1][:, s2 * 512 : (s2 + 1) * 512]
                )
                nc.gpsimd.collective_compute(
                    "AllGather",
                    mybir.AluOpType.bypass,
                    replica_groups=[[0, 1, 2, 3], [4, 5, 6, 7]],
                    ins=[cc_in1s[s2].opt()],
                    outs=[cc_out1s[s2].opt()],
                )
            for tkq in range(4):
                outproj_quarter(NS2 - 1, tkq)

    if split_waits:
        _split_multi_waits(nc)
    return nc


def _pack_x(xTb):
    """[DM, S] bf16 -> [128, NTS*KT*512] strip-major SBUF layout."""
    DM, S = xTb.shape
    KT, NTS = DM // 128, S // 512
    return np.ascontiguousarray(
        xTb.reshape(KT, 128, NTS, 512).transpose(1, 2, 0, 3).reshape(128, NTS * KT * 512)
    )


def _pack_w(wT):
    """[DM, C] -> [128, KT*C]: ktile-major free layout."""
    DM, C = wT.shape
    KT = DM // 128
    return np.ascontiguousarray(
        wT.reshape(KT, 128, C).transpose(1, 0, 2).reshape(128, KT * C)
    )


def make_in_maps(v, k, q, Wq, bqv, Wk, bkv, Wv, bvv, Wo, bov, S, DM, H, DK):
    """Per-core input dicts from full inputs (host prep: slice/transpose/cast/pack)."""
    bf16 = ml_dtypes.bfloat16
    HDK = H * DK
    DSL = HDK // GROUP
    xP = {}
    for b in range(2):
        xP[("q", b)] = _pack_x(np.ascontiguousarray(q[b].T).astype(bf16))
        xP[("k", b)] = _pack_x(np.ascontiguousarray(k[b].T).astype(bf16))
        xP[("v", b)] = _pack_x(np.ascontiguousarray(v[b].T).astype(bf16))
    WoT = np.ascontiguousarray(Wo.T)  # [HDK, HDK_out]
    in_maps = []
    for c in range(N_CORES):
        b = c // GROUP
        h0 = 4 * (c % GROUP)
        m = {
            "xqP": xP[("q", b)],
            "xkP": xP[("k", b)],
            "xvP": xP[("v", b)],
        }
        for kind, Wt, bt in (("k", Wk, bkv), ("v", Wv, bvv)):
            wp = np.empty((2, 128, DM // 128 * 128), np.float32)
            bp = np.empty((2, 128, 1), np.float32)
            for p in range(2):
                ha, hb = h0 + 2 * p, h0 + 2 * p + 1
                wT = np.empty((DM, 128), np.float32)
                wT[:, :64] = Wt[ha].T
                wT[:, 64:] = Wt[hb].T
                wp[p] = _pack_w(wT)
                bp[p, :64, 0] = bt[ha]
                bp[p, 64:, 0] = bt[hb]
            m[f"w{kind}P"] = wp.astype(bf16)
            if kind == "k":
                m["bk8"] = (bp / 8.0).astype(np.float32)
            else:
                m["bv"] = bp.astype(np.float32)
        # q: all 4 local heads side by side [DM, 256], packed
        wq4 = np.empty((DM, 256), np.float32)
        bqp = np.empty((2, 128, 1), np.float32)
        for hh in range(4):
            wq4[:, hh * 64 : (hh + 1) * 64] = Wq[h0 + hh].T
        for p in range(2):
            bqp[p, :64, 0] = bqv[h0 + 2 * p]
            bqp[p, 64:, 0] = bqv[h0 + 2 * p + 1]
        m["wq4P"] = _pack_w(wq4).astype(bf16)
        m["bq"] = bqp.astype(np.float32)
        d0 = DSL * (c % GROUP)
        m["woTP"] = _pack_w(np.ascontiguousarray(WoT[:, d0 : d0 + DSL])).astype(bf16)
        m["boT"] = np.ascontiguousarray(bov[d0 : d0 + DSL].reshape(2, 128).T).astype(
            np.float32
        )
        in_maps.append(m)
    return in_maps


def kernel(v, k, q, Wq, bq, Wk, bk, Wv, bv, Wo, bo, _trace=False):
    """Full inputs in, full output out. Runs the SPMD Bass kernel on 8 cores."""
    global last_results
    from concourse.bass_utils import run_bass_kernel_spmd

    v, k, q = (np.asarray(a, np.float32) for a in (v, k, q))
    B, S, DM = q.shape
    H, DK = Wq.shape[0], Wq.shape[1]
    HDK = H * DK
    DSL = HDK // GROUP

    nc = build_program(S=S, DM=DM, H=H, DK=DK)
    in_maps = make_in_maps(
        v,
        k,
        q,
        *(np.asarray(a, np.float32) for a in (Wq, bq, Wk, bk, Wv, bv, Wo, bo)),
        S=S,
        DM=DM,
        H=H,
        DK=DK,
    )
    res = run_bass_kernel_spmd(nc, in_maps, list(range(N_CORES)), trace=_trace)
    last_results = res
    out = np.empty((B, S, HDK), np.float32)
    for c in range(N_CORES):
        b = c // GROUP
        d0 = DSL * (c % GROUP)
        out[b, :, d0 : d0 + DSL] = res.results[c]["out"].T
    return out
